# revision 11
# baseline (speedup 1.0000x reference)
"""Trainium2 Bass kernel for nn_Decoder_36953898615460.

recon[B, D] = einsum('lbf,lfd->bd', acts[:n], W[:n]) + sum(bias[:n], 0)

Strategy (2-way F x 4-way B sharding, 8 NeuronCores):
  - Cores form 4 pairs (2p, 2p+1); pair p owns B block [p*512, (p+1)*512).
    Within a pair, core r%2 owns F half [r%2 * 3072, ...) -> local
    contraction K_loc = n*3072 (288 k-tiles at n=12).
  - fp16 inputs (host-side cast, free wrt HW time): halves DMA traffic vs
    fp32/fp32r; matmul rate is 1 cycle/row either way, so the ~373 us PE
    streaming floor (1728 matmuls x 512 cols @ 2.4 GHz) is unchanged.
  - Single full-K pass: 6 PSUM banks (one per 128-row D subtile of the
    [768, 512] partial) accumulate across all 24n k-tiles. Separate tiles
    per bank so the per-m drains don't serialize later matmuls.
  - Host prep is partition-major: a2[p, ko, b], w2[p, ko, d] so each chunk
    DMA is one contiguous multi-KB read per partition. Chunk schedule
    [2,2,4,8,8,...] k-tiles: tiny first chunks start the PE ~10 us sooner.
  - Pair reduction WITHOUT the collectives firmware (a 2-rank mesh RS
    still costs ~20 us of control-plane latency): each core drains its
    partial (+bias/2) to SBUF, then remote_dma_broadcast's its local rows
    384:768 straight into the peer's SBUF (relative dest (drid=0, dtpb=1)
    = tpb XOR 1, validated by probe), waits on the remote semaphore, and
    vector-adds recv + kept half -> y rows.
  - The even/odd asymmetry (who owns which true-D half) is folded into
    the HOST data layout: odd cores get W and bias with the two D halves
    swapped, so the kernel is exactly SPMD-uniform: keep local rows 0:384,
    send local rows 384:768. Even cores end up with true D rows 0:384 of
    their B block, odd cores with true rows 384:768.
  - Host: assemble 8x [384, 512] -> [768, 2048] -> transpose -> fp32.
"""

import numpy as np

import concourse.mybir as mybir
import concourse.tile as tile
from concourse import bacc
from concourse.bass import ts
from concourse.bass_utils import run_bass_kernel_spmd

NCORES = 8
B, F, D = 2048, 6144, 768
FSPLIT = 2
BSPLIT = 4
F_LOC = F // FSPLIT   # 3072
B_LOC = B // BSPLIT   # 512
P = 128
MD = D // P           # 6 d-subtiles
MH = MD // 2          # 3 kept / 3 sent
DR = D // FSPLIT      # 384 rows per core in the final output
CK = 8                # steady-state k-tiles (of 128) per DMA chunk
IN_DT = mybir.dt.float16
WIRE_DT = mybir.dt.float16

_nc_cache = {}
last_result = None  # BassKernelResults of the most recent run (for test harness)


def _build(n_layers: int):
    K_LOC = n_layers * F_LOC          # 36864 for n=12
    KT = K_LOC // P                   # 288 k-tiles
    chunk_sizes = [2, 2, 4] + [CK] * (KT // CK - 1)
    assert sum(chunk_sizes) == KT
    PAIRS = [[2 * p, 2 * p + 1] for p in range(BSPLIT)]
    HB = MH * B_LOC                   # 1536 columns per D-half

    nc = bacc.Bacc(None, num_devices=NCORES)
    a_ext = nc.dram_tensor("a2", [P, KT, B_LOC], IN_DT, kind="ExternalInput")
    w_ext = nc.dram_tensor("w2", [P, KT, D], IN_DT, kind="ExternalInput")
    b_ext = nc.dram_tensor("bias_t", [D, n_layers], mybir.dt.float32, kind="ExternalInput")
    y_ext = nc.dram_tensor("y", [DR, B_LOC], WIRE_DT, kind="ExternalOutput")

    b_v = b_ext[:, :].rearrange("(mo p) l -> p mo l", p=P)  # [128, 6, n]
    y_v = y_ext[:, :].rearrange("(mo p) b -> p mo b", p=P)  # [128, 3, 512]

    s_prep = nc.alloc_semaphore("s_prep")
    s_lsem = nc.alloc_semaphore("s_lsem")
    s_rsem = nc.alloc_semaphore("s_rsem")
    s_add = nc.alloc_semaphore("s_add")
    s_y = nc.alloc_semaphore("s_y")
    # Clear the hand-managed sems up front (raw, pre-tile; engines idle).
    # A peer's first remote inc can only arrive long after these retire:
    # it sends only after its own ~380 us compute + the barrier-AllGather.
    for s in (s_prep, s_lsem, s_rsem, s_add, s_y):
        nc.gpsimd.sem_clear(s)

    with tile.TileContext(nc) as tc:
        with (
            tc.tile_pool(name="apool", bufs=5) as apool,
            tc.tile_pool(name="wpool", bufs=5) as wpool,
            tc.tile_pool(name="cpool", bufs=1) as cpool,
            tc.tile_pool(name="pspool", bufs=1, space="PSUM") as pspool,
        ):
            # Persistent accumulators: one PSUM bank per D subtile.
            ps = [
                pspool.tile([P, B_LOC], mybir.dt.float32, tag=f"ps{m}", name=f"ps{m}")
                for m in range(MD)
            ]
            out_t = cpool.tile([P, MD * B_LOC], WIRE_DT)   # drained partial
            recv = cpool.tile([P, HB], WIRE_DT)            # peer's sent half
            y_t = cpool.tile([P, HB], WIRE_DT)             # final rows

            # Prefetch the first chunks before the (vector-only) bias prep
            # so input streaming owns the head of the sync DMA queue.
            chunks = []
            k0 = 0
            for ck in chunk_sizes[:4]:
                a_c = apool.tile([P, ck, B_LOC], IN_DT, tag=f"a{ck}")
                w_c = wpool.tile([P, ck, D], IN_DT, tag=f"w{ck}")
                nc.sync.dma_start(a_c[:], a_ext[:, k0 : k0 + ck, :])
                nc.sync.dma_start(w_c[:], w_ext[:, k0 : k0 + ck, :])
                chunks.append((a_c, w_c))
                k0 += ck

            # bias2[p, mo] = sum_l bias[l, mo*128+p] / FSPLIT
            bias_t = cpool.tile([P, MD, n_layers], mybir.dt.float32)
            nc.sync.dma_start(bias_t[:], b_v)
            bias2 = cpool.tile([P, MD], mybir.dt.float32)
            nc.vector.reduce_sum(bias2[:], bias_t[:], axis=mybir.AxisListType.X)
            nc.vector.tensor_scalar_mul(bias2[:], bias2[:], 1.0 / FSPLIT)

            NCH = len(chunk_sizes)
            k0 = 0
            for c, ck in enumerate(chunk_sizes):
                if c < len(chunks):
                    a_c, w_c = chunks[c]
                else:
                    a_c = apool.tile([P, ck, B_LOC], IN_DT, tag=f"a{ck}")
                    w_c = wpool.tile([P, ck, D], IN_DT, tag=f"w{ck}")
                    nc.sync.dma_start(a_c[:], a_ext[:, k0 : k0 + ck, :])
                    nc.sync.dma_start(w_c[:], w_ext[:, k0 : k0 + ck, :])
                if c < NCH - 1:
                    for k in range(ck):
                        first = c == 0 and k == 0
                        for m in range(MD):
                            nc.tensor.matmul(
                                ps[m][:],
                                w_c[:, k, ts(m, P)],
                                a_c[:, k],
                                start=first,
                                stop=False,
                            )
                else:
                    # Final chunk: m-outer, send half (m=3..5) first, so
                    # subtile m finishes and drains while m+1.. still stream.
                    for m in list(range(MH, MD)) + list(range(MH)):
                        for k in range(ck):
                            nc.tensor.matmul(
                                ps[m][:],
                                w_c[:, k, ts(m, P)],
                                a_c[:, k],
                                start=False,
                                stop=k == ck - 1,
                            )
                        nc.vector.tensor_scalar_add(
                            out_t[:, ts(m, B_LOC)], ps[m][:], bias2[:, m : m + 1]
                        )
                k0 += ck

            # Pair exchange: local rows 384:768 (out_t cols HB:2*HB) go to
            # the XOR-1 peer's recv; 8 slices across the 8 lane-pair slots
            # use all 16 SDMA engines. Each slice's dest rsem inc = 2.
            with tc.tile_critical():
                NSL = 8
                SL = HB // NSL  # 192 fp16 elems = 384 B per partition
                for sl in range(NSL):
                    rdests = [None] * NSL
                    rdests[sl] = (0, 1)
                    nc.gpsimd.remote_dma_broadcast(
                        recv[:, sl * SL : (sl + 1) * SL],
                        out_t[:, HB + sl * SL : HB + (sl + 1) * SL],
                        remote_sem=s_rsem,
                        local_sem=s_lsem,
                        rdests=rdests,
                    ).then_inc(s_prep, 1)
                # The critical entry is gated on a global-clock snapshot of
                # all prior tile work (incl. the drains), so the trigger's
                # deferred read of out_t is ordered. (An all_engine_barrier
                # here deadlocks inside tile_critical — probed.)
                nc.gpsimd.bir_kernel_barrier_wait(PAIRS)
                nc.gpsimd.wait_ge(s_prep, NSL)
                nc.gpsimd.trigger_dma(NSL)
                nc.vector.wait_ge(s_rsem, 2 * NSL)
                nc.vector.tensor_add(y_t[:], recv[:], out_t[:, 0:HB]).then_inc(
                    s_add, 1
                )
                nc.sync.wait_ge(s_add, 1)
                for mo in range(MH):
                    nc.sync.dma_start(y_v[:, mo], y_t[:, ts(mo, B_LOC)]).then_inc(
                        s_y, 16
                    )
                nc.sync.wait_ge(s_y, 16 * MH)
    nc.compile()
    return nc


def _get_nc(n_layers: int):
    if n_layers not in _nc_cache:
        _nc_cache[n_layers] = _build(n_layers)
    return _nc_cache[n_layers]


def kernel(acts: np.ndarray, W: np.ndarray, bias: np.ndarray, layer_idx) -> np.ndarray:
    global last_result
    n = int(layer_idx) + 1
    acts = np.asarray(acts, dtype=np.float32)[:n]  # [n, B, F]
    W = np.asarray(W, dtype=np.float32)[:n]        # [n, F, D]
    bias = np.asarray(bias, dtype=np.float32)[:n]  # [n, D]

    nc = _get_nc(n)

    KT = n * F_LOC // P
    FO = F_LOC // P  # 24 f-subtiles per core
    acts16 = acts.astype(np.float16)
    W16 = W.astype(np.float16)
    bias_t = np.ascontiguousarray(bias.T)  # [D, n] fp32
    # Odd cores see the two D halves swapped (their local rows 0:384 are
    # true rows 384:768), making the keep/send exchange SPMD-uniform.
    bias_t_odd = np.ascontiguousarray(np.concatenate([bias_t[DR:], bias_t[:DR]], axis=0))

    in_maps = []
    for r in range(NCORES):
        pair, fh = r // 2, r % 2
        b0, f0 = pair * B_LOC, fh * F_LOC
        # a2[p, (l, fo), b] = acts[l, b0+b, f0 + fo*128 + p]
        a2 = np.ascontiguousarray(
            acts16[:, b0 : b0 + B_LOC, f0 : f0 + F_LOC]
            .reshape(n, B_LOC, FO, P)
            .transpose(3, 0, 2, 1)
            .reshape(P, KT, B_LOC)
        )
        wr = W16[:, f0 : f0 + F_LOC, :]
        if fh == 1:
            wr = np.concatenate([wr[:, :, DR:], wr[:, :, :DR]], axis=2)
        # w2[p, (l, fo), d] = wr[l, fo*128 + p, d]
        w2 = np.ascontiguousarray(
            wr.reshape(n, FO, P, D).transpose(2, 0, 1, 3).reshape(P, KT, D)
        )
        in_maps.append(
            {"a2": a2, "w2": w2, "bias_t": bias_t if fh == 0 else bias_t_odd}
        )

    last_result = run_bass_kernel_spmd(nc, in_maps, core_ids=list(range(NCORES)))
    # Core 2p has true D rows [0, 384), core 2p+1 rows [384, 768) of block p.
    full = np.empty((D, B), dtype=np.float16)
    for r in range(NCORES):
        pair, fh = r // 2, r % 2
        full[fh * DR : (fh + 1) * DR, pair * B_LOC : (pair + 1) * B_LOC] = (
            last_result.results[r]["y"]
        )
    return full.T.astype(np.float32)  # [B, D] float32


# revision 16
# speedup vs baseline: 1.0188x; 1.0188x over previous
"""Trainium2 Bass kernel for nn_Decoder_36953898615460.

recon[B, D] = einsum('lbf,lfd->bd', acts[:n], W[:n]) + sum(bias[:n], 0)

Strategy (2-way F x 4-way B sharding, 8 NeuronCores):
  - Cores form 4 pairs (2p, 2p+1); pair p owns B block [p*512, (p+1)*512).
    Within a pair, core r%2 owns F half [r%2 * 3072, ...) -> local
    contraction K_loc = n*3072 (288 k-tiles at n=12).
  - fp16 inputs (host-side cast, free wrt HW time): halves DMA traffic vs
    fp32/fp32r; matmul rate is 1 cycle/row either way, so the ~373 us PE
    streaming floor (1728 matmuls x 512 cols @ 2.4 GHz) is unchanged.
  - Single full-K pass: 6 PSUM banks (one per 128-row D subtile of the
    [768, 512] partial) accumulate across all 24n k-tiles. Separate tiles
    per bank so the per-m drains don't serialize later matmuls.
  - Host prep is partition-major: a2[p, ko, b], w2[p, ko, d] so each chunk
    DMA is one contiguous multi-KB read per partition. Chunk schedule
    [2,2,4,8,8,...] k-tiles: tiny first chunks start the PE ~10 us sooner.
  - Pair reduction WITHOUT the collectives firmware (a 2-rank mesh RS
    still costs ~20 us of control-plane latency): each core drains its
    partial (+bias/2) to SBUF, then remote_dma_broadcast's its local rows
    384:768 straight into the peer's SBUF (relative dest (drid=0, dtpb=1)
    = tpb XOR 1, validated by probe), waits on the remote semaphore, and
    vector-adds recv + kept half -> y rows.
  - The even/odd asymmetry (who owns which true-D half) is folded into
    the HOST data layout: odd cores get W and bias with the two D halves
    swapped, so the kernel is exactly SPMD-uniform: keep local rows 0:384,
    send local rows 384:768. Even cores end up with true D rows 0:384 of
    their B block, odd cores with true rows 384:768.
  - Host: assemble 8x [384, 512] -> [768, 2048] -> transpose -> fp32.
"""

import numpy as np

import concourse.mybir as mybir
import concourse.tile as tile
from concourse import bacc
from concourse.bass import ts
from concourse.bass_utils import run_bass_kernel_spmd

NCORES = 8
B, F, D = 2048, 6144, 768
FSPLIT = 2
BSPLIT = 4
F_LOC = F // FSPLIT   # 3072
B_LOC = B // BSPLIT   # 512
P = 128
MD = D // P           # 6 d-subtiles
MH = MD // 2          # 3 kept / 3 sent
DR = D // FSPLIT      # 384 rows per core in the final output
CK = 8                # steady-state k-tiles (of 128) per DMA chunk
IN_DT = mybir.dt.float16
WIRE_DT = mybir.dt.float16

_nc_cache = {}
last_result = None  # BassKernelResults of the most recent run (for test harness)


def _build(n_layers: int):
    K_LOC = n_layers * F_LOC          # 36864 for n=12
    KT = K_LOC // P                   # 288 k-tiles
    chunk_sizes = [2, 2, 4] + [CK] * (KT // CK - 1)
    assert sum(chunk_sizes) == KT
    PAIRS = [[2 * p, 2 * p + 1] for p in range(BSPLIT)]
    HB = MH * B_LOC                   # 1536 columns per D-half

    nc = bacc.Bacc(None, num_devices=NCORES)
    a_ext = nc.dram_tensor("a2", [P, KT, B_LOC], IN_DT, kind="ExternalInput")
    w_ext = nc.dram_tensor("w2", [P, KT, D], IN_DT, kind="ExternalInput")
    b_ext = nc.dram_tensor("bias_t", [D, n_layers], mybir.dt.float32, kind="ExternalInput")
    y_ext = nc.dram_tensor("y", [DR, B_LOC], WIRE_DT, kind="ExternalOutput")

    b_v = b_ext[:, :].rearrange("(mo p) l -> p mo l", p=P)  # [128, 6, n]
    y_v = y_ext[:, :].rearrange("(mo p) b -> p mo b", p=P)  # [128, 3, 512]

    s_prep = nc.alloc_semaphore("s_prep")
    s_lsem = nc.alloc_semaphore("s_lsem")
    s_rsem = nc.alloc_semaphore("s_rsem")
    s_warm = nc.alloc_semaphore("s_warm")
    # Clear the hand-managed sems up front (raw, pre-tile; engines idle).
    # A peer's first remote inc can only arrive long after these retire:
    # its warm send follows its own barrier-gated prelude.
    for s in (s_prep, s_lsem, s_rsem, s_warm):
        nc.gpsimd.sem_clear(s)

    with tile.TileContext(nc) as tc:
        with (
            tc.tile_pool(name="apool", bufs=5) as apool,
            tc.tile_pool(name="wpool", bufs=5) as wpool,
            tc.tile_pool(name="cpool", bufs=1) as cpool,
            tc.tile_pool(name="pspool", bufs=1, space="PSUM") as pspool,
        ):
            # Persistent accumulators: one PSUM bank per D subtile.
            ps = [
                pspool.tile([P, B_LOC], mybir.dt.float32, tag=f"ps{m}", name=f"ps{m}")
                for m in range(MD)
            ]
            out_t = cpool.tile([P, MD * B_LOC], WIRE_DT)   # drained partial
            recv = cpool.tile([P, HB], WIRE_DT)            # peer's sent half
            y_t = cpool.tile([P, HB], WIRE_DT)             # final rows
            warm_t = cpool.tile([P, 16], mybir.dt.float32)  # warm-up send pad

            # Prefetch the first chunks before the (vector-only) bias prep
            # so input streaming owns the head of the sync DMA queue.
            chunks = []
            k0 = 0
            for ck in chunk_sizes[:4]:
                a_c = apool.tile([P, ck, B_LOC], IN_DT, tag=f"a{ck}")
                w_c = wpool.tile([P, ck, D], IN_DT, tag=f"w{ck}")
                nc.sync.dma_start(a_c[:], a_ext[:, k0 : k0 + ck, :])
                nc.sync.dma_start(w_c[:], w_ext[:, k0 : k0 + ck, :])
                chunks.append((a_c, w_c))
                k0 += ck

            # bias2[p, mo] = sum_l bias[l, mo*128+p] / FSPLIT
            bias_t = cpool.tile([P, MD, n_layers], mybir.dt.float32)
            nc.sync.dma_start(bias_t[:], b_v)
            bias2 = cpool.tile([P, MD], mybir.dt.float32)
            nc.vector.reduce_sum(bias2[:], bias_t[:], axis=mybir.AxisListType.X)
            nc.vector.tensor_scalar_mul(bias2[:], bias2[:], 1.0 / FSPLIT)

            # Early critical: generate the exchange descriptors (Q7 descgen,
            # ~1 us each + a ucode library reload) while the PE streams, and
            # fire a tiny warm-up remote send to heat the SWDGE trigger ->
            # SDMA-fetch path. Sources are read at *trigger* time, so the
            # real slices are safe to prep before out_t is written.
            # no_gpsimd_drain: the exit's gpsimd drain would block on the 8
            # still-untriggered preps in the SWDGE ring (trigger comes at the
            # tail) — deadlock. Ordering to the trigger is via s_prep.
            NSL = 8
            SL = HB // NSL  # 192 fp16 elems = 384 B per partition
            with tc.tile_critical(no_gpsimd_drain=True):
                nc.gpsimd.remote_dma_broadcast(
                    warm_t[:],
                    warm_t[:],
                    remote_sem=s_warm,
                    local_sem=s_lsem,
                    rdests=[(0, 1)] + [None] * (NSL - 1),
                ).then_inc(s_prep, 1)
                nc.gpsimd.wait_ge(s_prep, 1)
                nc.gpsimd.trigger_dma(1)
                for sl in range(NSL):
                    rdests = [None] * NSL
                    rdests[sl] = (0, 1)
                    nc.gpsimd.remote_dma_broadcast(
                        recv[:, sl * SL : (sl + 1) * SL],
                        out_t[:, HB + sl * SL : HB + (sl + 1) * SL],
                        remote_sem=s_rsem,
                        local_sem=s_lsem,
                        rdests=rdests,
                    ).then_inc(s_prep, 1)

            NCH = len(chunk_sizes)
            k0 = 0
            for c, ck in enumerate(chunk_sizes):
                if c < len(chunks):
                    a_c, w_c = chunks[c]
                else:
                    a_c = apool.tile([P, ck, B_LOC], IN_DT, tag=f"a{ck}")
                    w_c = wpool.tile([P, ck, D], IN_DT, tag=f"w{ck}")
                    nc.sync.dma_start(a_c[:], a_ext[:, k0 : k0 + ck, :])
                    nc.sync.dma_start(w_c[:], w_ext[:, k0 : k0 + ck, :])
                if c < NCH - 1:
                    for k in range(ck):
                        first = c == 0 and k == 0
                        for m in range(MD):
                            nc.tensor.matmul(
                                ps[m][:],
                                w_c[:, k, ts(m, P)],
                                a_c[:, k],
                                start=first,
                                stop=False,
                            )
                else:
                    # Final chunk: m-outer, send half (m=3..5) first, so
                    # subtile m finishes and drains while m+1.. still stream.
                    for m in list(range(MH, MD)) + list(range(MH)):
                        for k in range(ck):
                            nc.tensor.matmul(
                                ps[m][:],
                                w_c[:, k, ts(m, P)],
                                a_c[:, k],
                                start=False,
                                stop=k == ck - 1,
                            )
                        nc.vector.tensor_scalar_add(
                            out_t[:, ts(m, B_LOC)], ps[m][:], bias2[:, m : m + 1]
                        )
                k0 += ck

            # Pair exchange: local rows 384:768 (out_t cols HB:2*HB) go to
            # the XOR-1 peer's recv; 8 slices across the 8 lane-pair slots
            # use all 16 SDMA engines. Each slice's dest rsem inc = 2.
            # The critical entry is gated on a global-clock snapshot of all
            # prior tile work (incl. the drains), so the trigger's deferred
            # read of out_t is ordered. (An all_engine_barrier here
            # deadlocks inside tile_critical — probed.)
            with tc.tile_critical():
                nc.gpsimd.bir_kernel_barrier_wait(PAIRS)
                nc.gpsimd.wait_ge(s_prep, NSL + 1)
                nc.gpsimd.trigger_dma(NSL)
                nc.vector.wait_ge(s_rsem, 2 * NSL)
                nc.vector.tensor_add(y_t[:], recv[:], out_t[:, 0:HB])
            # Tile-tracked output writes, ordered after the critical exit;
            # the tile epilogue tracks their completion.
            for mo in range(MH):
                nc.sync.dma_start(y_v[:, mo], y_t[:, ts(mo, B_LOC)])
    nc.compile()
    return nc


def _get_nc(n_layers: int):
    if n_layers not in _nc_cache:
        _nc_cache[n_layers] = _build(n_layers)
    return _nc_cache[n_layers]


def kernel(acts: np.ndarray, W: np.ndarray, bias: np.ndarray, layer_idx) -> np.ndarray:
    global last_result
    n = int(layer_idx) + 1
    acts = np.asarray(acts, dtype=np.float32)[:n]  # [n, B, F]
    W = np.asarray(W, dtype=np.float32)[:n]        # [n, F, D]
    bias = np.asarray(bias, dtype=np.float32)[:n]  # [n, D]

    nc = _get_nc(n)

    KT = n * F_LOC // P
    FO = F_LOC // P  # 24 f-subtiles per core
    acts16 = acts.astype(np.float16)
    W16 = W.astype(np.float16)
    bias_t = np.ascontiguousarray(bias.T)  # [D, n] fp32
    # Odd cores see the two D halves swapped (their local rows 0:384 are
    # true rows 384:768), making the keep/send exchange SPMD-uniform.
    bias_t_odd = np.ascontiguousarray(np.concatenate([bias_t[DR:], bias_t[:DR]], axis=0))

    in_maps = []
    for r in range(NCORES):
        pair, fh = r // 2, r % 2
        b0, f0 = pair * B_LOC, fh * F_LOC
        # a2[p, (l, fo), b] = acts[l, b0+b, f0 + fo*128 + p]
        a2 = np.ascontiguousarray(
            acts16[:, b0 : b0 + B_LOC, f0 : f0 + F_LOC]
            .reshape(n, B_LOC, FO, P)
            .transpose(3, 0, 2, 1)
            .reshape(P, KT, B_LOC)
        )
        wr = W16[:, f0 : f0 + F_LOC, :]
        if fh == 1:
            wr = np.concatenate([wr[:, :, DR:], wr[:, :, :DR]], axis=2)
        # w2[p, (l, fo), d] = wr[l, fo*128 + p, d]
        w2 = np.ascontiguousarray(
            wr.reshape(n, FO, P, D).transpose(2, 0, 1, 3).reshape(P, KT, D)
        )
        in_maps.append(
            {"a2": a2, "w2": w2, "bias_t": bias_t if fh == 0 else bias_t_odd}
        )

    last_result = run_bass_kernel_spmd(nc, in_maps, core_ids=list(range(NCORES)))
    # Core 2p has true D rows [0, 384), core 2p+1 rows [384, 768) of block p.
    full = np.empty((D, B), dtype=np.float16)
    for r in range(NCORES):
        pair, fh = r // 2, r % 2
        full[fh * DR : (fh + 1) * DR, pair * B_LOC : (pair + 1) * B_LOC] = (
            last_result.results[r]["y"]
        )
    return full.T.astype(np.float32)  # [B, D] float32


# revision 22
# speedup vs baseline: 1.0658x; 1.0461x over previous
"""Trainium2 Bass kernel for nn_Decoder_36953898615460.

recon[B, D] = einsum('lbf,lfd->bd', acts[:n], W[:n]) + sum(bias[:n], 0)

Strategy (2-way F x 4-way B sharding, 8 NeuronCores):
  - Cores form 4 pairs (2p, 2p+1); pair p owns B block [p*512, (p+1)*512).
    Within a pair, core r%2 owns F half [r%2 * 3072, ...) -> local
    contraction K_loc = n*3072 (288 k-tiles at n=12).
  - fp16 inputs (host-side cast, free wrt HW time): halves DMA traffic vs
    fp32/fp32r; matmul rate is 1 cycle/row either way, so the ~373 us PE
    streaming floor (1728 matmuls x 512 cols @ 2.4 GHz) is unchanged.
  - Single full-K pass: 6 PSUM banks (one per 128-row D subtile of the
    [768, 512] partial) accumulate across all 24n k-tiles. Separate tiles
    per bank so the per-m drains don't serialize later matmuls.
  - Host prep is partition-major: a2[p, ko, b], w2[p, ko, d] so each chunk
    DMA is one contiguous multi-KB read per partition. Chunk schedule
    [2,2,4,8,8,...] k-tiles: tiny first chunks start the PE ~10 us sooner.
  - Pair reduction via a 2-rank ReduceScatter per pair (replica groups
    [[0,1],[2,3],[4,5],[6,7]]), fp16 wire. A tiny warm-up RS at kernel
    start absorbs the ~11 us ncfw wake latency and aligns the pair, so
    the real RS begins ~1 us after trigger (measured; its EVENT waits
    drop from ~27 us to ~0.1 us). An SBUF->SBUF remote_dma exchange was
    tried instead and measured SLOWER (~5.6 us per dummy descriptor on
    the remote path serializes the slices).
  - Host: assemble 8x [384, 512] -> [768, 2048] -> transpose -> fp32.
"""

import numpy as np

import concourse.mybir as mybir
import concourse.tile as tile
from concourse import bacc
from concourse.bass import ts
from concourse.bass_utils import run_bass_kernel_spmd

NCORES = 8
B, F, D = 2048, 6144, 768
FSPLIT = 2
BSPLIT = 4
F_LOC = F // FSPLIT   # 3072
B_LOC = B // BSPLIT   # 512
P = 128
MD = D // P           # 6 d-subtiles
MH = MD // 2          # 3 kept / 3 sent
DR = D // FSPLIT      # 384 rows per core in the final output
CK = 8                # steady-state k-tiles (of 128) per DMA chunk
IN_DT = mybir.dt.float16
WIRE_DT = mybir.dt.float16

_nc_cache = {}
last_result = None  # BassKernelResults of the most recent run (for test harness)


def _build(n_layers: int):
    K_LOC = n_layers * F_LOC          # 36864 for n=12
    KT = K_LOC // P                   # 288 k-tiles
    chunk_sizes = [2, 2, 4] + [CK] * (KT // CK - 1)
    assert sum(chunk_sizes) == KT
    PAIRS = [[2 * p, 2 * p + 1] for p in range(BSPLIT)]
    HB = MH * B_LOC                   # 1536 columns per D-half

    nc = bacc.Bacc(None, num_devices=NCORES)
    a_ext = nc.dram_tensor("a2", [P, KT, B_LOC], IN_DT, kind="ExternalInput")
    w_ext = nc.dram_tensor("w2", [P, KT, D], IN_DT, kind="ExternalInput")
    b_ext = nc.dram_tensor("bias_t", [D, n_layers], mybir.dt.float32, kind="ExternalInput")
    y_ext = nc.dram_tensor("y", [DR, B_LOC], WIRE_DT, kind="ExternalOutput")

    partial = nc.dram_tensor("partial", [D, B_LOC], WIRE_DT)
    reduced = nc.dram_tensor("reduced", [DR, B_LOC], WIRE_DT)
    warm_in = nc.dram_tensor("warm_in", [2, 16], mybir.dt.float32)
    warm_out = nc.dram_tensor("warm_out", [1, 16], mybir.dt.float32)

    b_v = b_ext[:, :].rearrange("(mo p) l -> p mo l", p=P)        # [128, 6, n]
    partial_v = partial[:, :].rearrange("(mo p) b -> p mo b", p=P)  # [128, 6, 512]

    with tile.TileContext(nc) as tc:
        with (
            tc.tile_pool(name="apool", bufs=5) as apool,
            tc.tile_pool(name="wpool", bufs=5) as wpool,
            tc.tile_pool(name="cpool", bufs=1) as cpool,
            tc.tile_pool(name="pspool", bufs=1, space="PSUM") as pspool,
        ):
            # Persistent accumulators: one PSUM bank per D subtile.
            ps = [
                pspool.tile([P, B_LOC], mybir.dt.float32, tag=f"ps{m}", name=f"ps{m}")
                for m in range(MD)
            ]
            out_t = cpool.tile([P, MD * B_LOC], WIRE_DT)   # drained partial

            # Prefetch the first chunks before the (vector-only) bias prep
            # so input streaming owns the head of the sync DMA queue.
            chunks = []
            k0 = 0
            for ck in chunk_sizes[:4]:
                a_c = apool.tile([P, ck, B_LOC], IN_DT, tag=f"a{ck}")
                w_c = wpool.tile([P, ck, D], IN_DT, tag=f"w{ck}")
                nc.sync.dma_start(a_c[:], a_ext[:, k0 : k0 + ck, :])
                nc.sync.dma_start(w_c[:], w_ext[:, k0 : k0 + ck, :])
                chunks.append((a_c, w_c))
                k0 += ck

            # bias2[p, mo] = sum_l bias[l, mo*128+p] / FSPLIT
            bias_t = cpool.tile([P, MD, n_layers], mybir.dt.float32)
            nc.sync.dma_start(bias_t[:], b_v)
            bias2 = cpool.tile([P, MD], mybir.dt.float32)
            nc.vector.reduce_sum(bias2[:], bias_t[:], axis=mybir.AxisListType.X)
            nc.vector.tensor_scalar_mul(bias2[:], bias2[:], 1.0 / FSPLIT)

            # Warm up the collective firmware while the PE streams: the
            # first collective in a NEFF pays ~11 us of ncfw wake latency
            # plus ~35 us of per-event skew; a warmed second one pays ~1 us.
            nc.gpsimd.collective_compute(
                "ReduceScatter",
                mybir.AluOpType.add,
                replica_groups=PAIRS,
                ins=[warm_in[:, :].opt()],
                outs=[warm_out[:, :].opt()],
            )

            NCH = len(chunk_sizes)
            k0 = 0
            for c, ck in enumerate(chunk_sizes):
                if c < len(chunks):
                    a_c, w_c = chunks[c]
                else:
                    a_c = apool.tile([P, ck, B_LOC], IN_DT, tag=f"a{ck}")
                    w_c = wpool.tile([P, ck, D], IN_DT, tag=f"w{ck}")
                    nc.sync.dma_start(a_c[:], a_ext[:, k0 : k0 + ck, :])
                    nc.sync.dma_start(w_c[:], w_ext[:, k0 : k0 + ck, :])
                if c < NCH - 1:
                    for k in range(ck):
                        first = c == 0 and k == 0
                        for m in range(MD):
                            nc.tensor.matmul(
                                ps[m][:],
                                w_c[:, k, ts(m, P)],
                                a_c[:, k],
                                start=first,
                                stop=False,
                            )
                else:
                    # Final chunk: m-outer, so subtile m finishes, drains,
                    # and writes its partial slice while m+1.. still stream.
                    for m in range(MD):
                        for k in range(ck):
                            nc.tensor.matmul(
                                ps[m][:],
                                w_c[:, k, ts(m, P)],
                                a_c[:, k],
                                start=False,
                                stop=k == ck - 1,
                            )
                        nc.vector.tensor_scalar_add(
                            out_t[:, ts(m, B_LOC)], ps[m][:], bias2[:, m : m + 1]
                        )
                        nc.sync.dma_start(partial_v[:, m], out_t[:, ts(m, B_LOC)])
                k0 += ck

            nc.gpsimd.collective_compute(
                "ReduceScatter",
                mybir.AluOpType.add,
                replica_groups=PAIRS,
                ins=[partial[:, :].opt()],
                outs=[reduced[:, :].opt()],
            )
            # y on the (idle) sync HWDGE queue: the gpsimd SWDGE path costs
            # an extra ~4 us queue drain in the tile epilogue.
            nc.sync.dma_start(y_ext[:, :], reduced[:, :])
    nc.compile()
    return nc


def _get_nc(n_layers: int):
    if n_layers not in _nc_cache:
        _nc_cache[n_layers] = _build(n_layers)
    return _nc_cache[n_layers]


def kernel(acts: np.ndarray, W: np.ndarray, bias: np.ndarray, layer_idx) -> np.ndarray:
    global last_result
    n = int(layer_idx) + 1
    acts = np.asarray(acts, dtype=np.float32)[:n]  # [n, B, F]
    W = np.asarray(W, dtype=np.float32)[:n]        # [n, F, D]
    bias = np.asarray(bias, dtype=np.float32)[:n]  # [n, D]

    nc = _get_nc(n)

    KT = n * F_LOC // P
    FO = F_LOC // P  # 24 f-subtiles per core
    acts16 = acts.astype(np.float16)
    W16 = W.astype(np.float16)
    bias_t = np.ascontiguousarray(bias.T)  # [D, n] fp32, same on every core

    in_maps = []
    for r in range(NCORES):
        pair, fh = r // 2, r % 2
        b0, f0 = pair * B_LOC, fh * F_LOC
        # a2[p, (l, fo), b] = acts[l, b0+b, f0 + fo*128 + p]
        a2 = np.ascontiguousarray(
            acts16[:, b0 : b0 + B_LOC, f0 : f0 + F_LOC]
            .reshape(n, B_LOC, FO, P)
            .transpose(3, 0, 2, 1)
            .reshape(P, KT, B_LOC)
        )
        # w2[p, (l, fo), d] = W[l, f0 + fo*128 + p, d]
        w2 = np.ascontiguousarray(
            W16[:, f0 : f0 + F_LOC, :]
            .reshape(n, FO, P, D)
            .transpose(2, 0, 1, 3)
            .reshape(P, KT, D)
        )
        in_maps.append({"a2": a2, "w2": w2, "bias_t": bias_t})

    last_result = run_bass_kernel_spmd(nc, in_maps, core_ids=list(range(NCORES)))
    # Core 2p has true D rows [0, 384), core 2p+1 rows [384, 768) of block p.
    full = np.empty((D, B), dtype=np.float16)
    for r in range(NCORES):
        pair, fh = r // 2, r % 2
        full[fh * DR : (fh + 1) * DR, pair * B_LOC : (pair + 1) * B_LOC] = (
            last_result.results[r]["y"]
        )
    return full.T.astype(np.float32)  # [B, D] float32
